# revision 1
# baseline (speedup 1.0000x reference)
"""Trainium2 Bass kernel for nn_AssignAttention (softmax over the query axis).

Math (per batch b):
  q = (query @ Wq)  [N, C] -> heads [N, H, hd]
  k = (key   @ Wk)  [S, C] -> heads [S, H, hd]
  raw[h, n, s] = (q_h @ k_h^T) * hd^-0.5
  attn = softmax(raw, axis=n)                  # normalize over queries, per (h, s)
  attn = attn / max(sum_s attn, 1)             # clamp-normalize over s, per (h, n)
  out[n, h*hd:  ] = sum_s attn[h, n, s] * key[s, h*hd: (h+1)*hd]
  returns (out, out_style) with out_style == out

Distribution: data-parallel over B=16 across 8 NeuronCores (2 batches/core).

v2 structure (vs baseline):
  - key/query transposes via DMA XBAR (dma_start transpose=True) instead of
    PE transposes: frees the PE and a PSUM bank.
  - exp batched: 2 ACT instructions per s-tile ([128, 1024] each, 4 heads)
    instead of 8 ([128,256]) + 8 accumulator reads.
  - softmax denominator D[s,h] via segmented DVE tensor_reduce
    [128, 4, 256] -> [128, 4] from the bf16 e tile.
  - vaug (v rows scaled by 1/D) in ONE tensor_tensor broadcast mul per tile.
  - scores issued in order (0,3,2,1,4,7,6,5) so concurrently-paired MMs
    (row groups 0-63 vs 64-127) write different PSUM banks.
"""

import os
import threading

import numpy as np

B, N, S, C, H = 16, 256, 4096, 512, 8
HD = C // H
NCORES = 8
BL = B // NCORES  # batches per core
SCALE = float(HD) ** -0.5

_cache = {}
_lock = threading.Lock()


def _build():
    from contextlib import ExitStack

    import concourse.bass as bass
    import concourse.tile as tile
    from concourse import bacc, mybir
    from concourse.masks import make_identity

    f32 = mybir.dt.float32
    bf16 = mybir.dt.bfloat16

    nc = bacc.Bacc(
        "TRN2",
        target_bir_lowering=False,
        debug=False,
        enable_asserts=False,
        num_devices=NCORES,
    )
    q_ap = nc.dram_tensor("query", [BL, N, C], f32, kind="ExternalInput").ap()
    k_ap = nc.dram_tensor("key", [BL, S, C], f32, kind="ExternalInput").ap()
    wq_ap = nc.dram_tensor("Wq", [C, C], f32, kind="ExternalInput").ap()
    wk_ap = nc.dram_tensor("Wk", [C, C], f32, kind="ExternalInput").ap()
    out_ap = nc.dram_tensor("out", [BL, N, C], f32, kind="ExternalOutput").ap()
    out2_ap = nc.dram_tensor("out_style", [BL, N, C], f32, kind="ExternalOutput").ap()
    DEBUG = bool(int(os.environ.get("K_DEBUG", "0")))
    if DEBUG:
        dbg_qtp = nc.dram_tensor("dbg_qtp", [128, 4 * N], f32, kind="ExternalOutput").ap()
        dbg_ktpj = nc.dram_tensor("dbg_ktpj", [128, 4 * S], f32, kind="ExternalOutput").ap()
        dbg_et = nc.dram_tensor("dbg_et", [128, H * N], f32, kind="ExternalOutput").ap()
        dbg_den = nc.dram_tensor("dbg_den", [128, H], f32, kind="ExternalOutput").ap()

    NT = S // 128          # 32 s-tiles of 128
    NJ = S // 512          # 8 macro chunks of 512 rows
    NCK = C // 128         # 4 c_in chunks
    NM = C // 128          # 4 c_out chunks

    # Head h's scores live at free slot SLOT[h] (bank = slot//2) of the sc
    # tile. Heads sharing a PSUM bank must use the SAME PE row group (base
    # partition) -- mixed row-groups writing one bank is an
    # NRT_EXEC_UNIT_UNRECOVERABLE device crash. Banks hold same-parity pairs
    # (0,2), (1,3), (4,6), (5,7); issuing h=0..7 in order then makes each
    # consecutive MM pair (even, odd) hit different banks AND different row
    # groups, so they can run concurrently.
    SLOT = [(h % 2) * 2 + (h // 2) % 2 + (h // 4) * 4 for h in range(H)]

    with tile.TileContext(nc) as tc, ExitStack() as ctx:
        const = ctx.enter_context(tc.tile_pool(name="const", bufs=1))
        # weights, bf16, layout [c_in_chunk(part=128), k*C + c_out]
        wq_bf = const.tile([128, NCK * C], bf16)
        wk_bf = const.tile([128, NCK * C], bf16)
        nc.gpsimd.dma_start(
            wq_bf[:].rearrange("p (k c) -> p k c", k=NCK),
            wq_ap.rearrange("(k p) c -> p k c", k=NCK),
        )
        nc.gpsimd.dma_start(
            wk_bf[:].rearrange("p (k c) -> p k c", k=NCK),
            wk_ap.rearrange("(k p) c -> p k c", k=NCK),
        )
        ident = const.tile([128, 128], bf16)
        make_identity(nc, ident[:])

        # SBUF pools
        kb_pool = ctx.enter_context(tc.tile_pool(name="kb", bufs=2))
        ktp_pool = ctx.enter_context(tc.tile_pool(name="ktp", bufs=3))
        qpool = ctx.enter_context(tc.tile_pool(name="qpool", bufs=2))
        ktpj_pool = ctx.enter_context(tc.tile_pool(name="ktpj", bufs=2))
        epool = ctx.enter_context(tc.tile_pool(name="epool", bufs=4))
        spool = ctx.enter_context(tc.tile_pool(name="spool", bufs=4))
        opool = ctx.enter_context(tc.tile_pool(name="opool", bufs=2))

        # PSUM pools (8 banks: trp 1 + kprj 1 + sc 4 + oacc 2).
        # dacc is a per-t tile aliasing the sc pool's banks.
        trp_pool = ctx.enter_context(tc.tile_pool(name="trp", bufs=1, space="PSUM"))
        kprj_pool = ctx.enter_context(tc.tile_pool(name="kprj", bufs=1, space="PSUM"))
        sc_pool = ctx.enter_context(tc.tile_pool(name="sc", bufs=1, space="PSUM"))
        oacc_pool = ctx.enter_context(tc.tile_pool(name="oacc", bufs=1, space="PSUM"))

        for b in range(BL):
            # ---------------- Stage A: q path ----------------
            qf_bf = qpool.tile([128, 2 * C], bf16, tag="qf")
            nc.gpsimd.dma_start(
                qf_bf[:].rearrange("p (j c) -> p j c", j=2),
                q_ap[b].rearrange("(j p) c -> p j c", j=2),
            )
            # transpose query -> qT [c(part, by chunk), n]
            qt_sb = qpool.tile([128, NCK * N], bf16, tag="qt")
            for j in range(2):
                tp = trp_pool.tile([128, 1024], bf16, tag="trp")
                for ck in range(NCK):
                    nc.tensor.transpose(
                        tp[:, ck * 128 : (ck + 1) * 128],
                        qf_bf[:, j * C + ck * 128 : j * C + (ck + 1) * 128],
                        ident[:],
                    )
                for ck in range(NCK):
                    nc.vector.tensor_copy(
                        qt_sb[:, ck * N + j * 128 : ck * N + j * 128 + 128],
                        tp[:, ck * 128 : (ck + 1) * 128],
                    )
            # q projection (transposed out): qTp [c_out(part by chunk m), n]
            qtp = qpool.tile([128, NM * N], bf16, tag="qtp")
            for m in range(NM):
                pq = kprj_pool.tile([128, 512], f32, tag="kprj")
                for k in range(NCK):
                    nc.tensor.matmul(
                        pq[:, :N],
                        lhsT=wq_bf[:, k * C + m * 128 : k * C + (m + 1) * 128],
                        rhs=qt_sb[:, k * N : (k + 1) * N],
                        start=(k == 0),
                        stop=(k == NCK - 1),
                    )
                nc.vector.tensor_copy(qtp[:, m * N : (m + 1) * N], pq[:, :N])

            # ---------------- Stage B: k path ----------------
            kb = kb_pool.tile([128, NT * C], bf16, tag="kb")  # natural [s, c] (= V)
            ktpj = ktpj_pool.tile([128, NM * S], bf16, tag="ktpj")  # kT proj [c_out, s]
            for j in range(NJ):
                # load 512 rows of key, cast f32->bf16 during DMA
                nc.gpsimd.dma_start(
                    kb[:, 4 * j * C : 4 * (j + 1) * C].rearrange(
                        "p (t c) -> p t c", t=4
                    ),
                    k_ap[b, j * 512 : (j + 1) * 512, :].rearrange(
                        "(t p) c -> p t c", t=4
                    ),
                )
                # transpose to keyT chunks -> ktin[:, ck*512 + tt*128]
                ktin = ktp_pool.tile([128, NCK * 512], bf16, tag="ktin")
                for ckp in range(2):
                    tp = trp_pool.tile([128, 1024], bf16, tag="trp")
                    for tt in range(4):
                        t = 4 * j + tt
                        for cc in range(2):
                            ck = ckp * 2 + cc
                            nc.tensor.transpose(
                                tp[:, cc * 512 + tt * 128 : cc * 512 + tt * 128 + 128],
                                kb[:, t * C + ck * 128 : t * C + (ck + 1) * 128],
                                ident[:],
                            )
                    nc.vector.tensor_copy(
                        ktin[:, ckp * 1024 : (ckp + 1) * 1024], tp[:]
                    )
                # k projection, transposed output [c_out(part), s]
                for m in range(NM):
                    pk = kprj_pool.tile([128, 512], f32, tag="kprj")
                    for k in range(NCK):
                        nc.tensor.matmul(
                            pk[:],
                            lhsT=wk_bf[:, k * C + m * 128 : k * C + (m + 1) * 128],
                            rhs=ktin[:, k * 512 : (k + 1) * 512],
                            start=(k == 0),
                            stop=(k == NCK - 1),
                        )
                    if m % 2 == 0:
                        nc.vector.tensor_copy(
                            ktpj[:, m * S + j * 512 : m * S + (j + 1) * 512], pk[:]
                        )
                    else:
                        nc.scalar.copy(
                            ktpj[:, m * S + j * 512 : m * S + (j + 1) * 512], pk[:]
                        )

            # ---------------- Stage C: attention ----------------
            oacc = oacc_pool.tile([128, 16 * HD], f32, tag="oacc")
            divs = spool.tile([128, 16], f32, tag="divs")
            nc.vector.memset(divs[:], 0.0)

            # scores for tile t: two 2-bank PSUM sets (4 heads each).
            # Within a set, each bank holds a same-parity head pair
            # ((0,2)/(1,3)): mixed row-groups in one bank is a device crash.
            # Issue order (0,1,2,3) makes consecutive (even,odd) MMs hit
            # different banks AND row groups -> they run concurrently.
            def scores(t):
                sets = []
                for half in range(2):
                    scp = sc_pool.tile([128, 4 * N], f32, tag=f"sc{half}")
                    sets.append(scp)
                    for hh in range(4):
                        h = half * 4 + hh
                        m, hp = h // 2, (h % 2) * 64
                        s0 = ((hh % 2) * 2 + hh // 2) * N
                        nc.tensor.matmul(
                            scp[:, s0 : s0 + N],
                            lhsT=ktpj[
                                hp : hp + 64, m * S + t * 128 : m * S + t * 128 + 128
                            ],
                            rhs=qtp[hp : hp + 64, m * N : (m + 1) * N],
                            start=True,
                            stop=True,
                        )
                return sets

            import contextlib

            prev_sets = scores(0)
            for t in range(NT):
                cur_sets = prev_sets
                if t + 1 < NT:
                    prev_sets = scores(t + 1)
                et = epool.tile([128, H * N], bf16, tag="et")
                den = spool.tile([128, H], f32, tag="den")
                rt = spool.tile([128, H], f32, tag="rt")
                rbf = spool.tile([128, H], bf16, tag="rbf")
                vaug = spool.tile([128, H * HD], bf16, tag="vaug")
                crit = (
                    tc.tile_critical()
                    if (t == 0 or t == NT - 1)
                    else contextlib.nullcontext()
                )
                for half in range(2):
                    h0 = half * 4
                    nc.scalar.activation(
                        et[:, half * 1024 : (half + 1) * 1024],
                        cur_sets[half][:],
                        mybir.ActivationFunctionType.Exp,
                        scale=SCALE,
                    )
                    nc.vector.tensor_reduce(
                        den[:, h0 : h0 + 4],
                        et[:, half * 1024 : (half + 1) * 1024].rearrange(
                            "p (h n) -> p h n", h=4
                        ),
                        mybir.AxisListType.X,
                        mybir.AluOpType.add,
                    )
                    nc.vector.reciprocal(rt[:, h0 : h0 + 4], den[:, h0 : h0 + 4])
                    # rt is slot-ordered; rbf is head-ordered (the slot
                    # permutation swaps the middle two of each group of 4).
                    nc.vector.tensor_copy(
                        rbf[:, h0 : h0 + 4],
                        rt[:, h0 : h0 + 4]
                        .rearrange("p (a b) -> p a b", a=2, b=2)
                        .transpose([0, 2, 1]),
                    )
                    # vaug[s, h, :] = v[s, h, :] * (1/D[s, h])
                    veng = nc.vector if half == 0 else nc.gpsimd
                    veng.tensor_tensor(
                        vaug[:, h0 * HD : (h0 + 4) * HD].rearrange(
                            "p (h c) -> p h c", h=4
                        ),
                        kb[:, t * C + h0 * HD : t * C + (h0 + 4) * HD].rearrange(
                            "p (h c) -> p h c", h=4
                        ),
                        rbf[:, h0 : h0 + 4, None].broadcast_to((128, 4, HD)),
                        mybir.AluOpType.mult,
                    )
                    with crit if half == 1 else contextlib.nullcontext():
                        for hh in range(4):
                            h = h0 + hh
                            for ncn in range(2):
                                g = h * 2 + ncn
                                s0 = half * 1024 + ((hh % 2) * 2 + hh // 2) * N
                                lhsT = et[:, s0 + ncn * 128 : s0 + ncn * 128 + 128]
                                nc.tensor.matmul(
                                    oacc[:, g * HD : (g + 1) * HD],
                                    lhsT=lhsT,
                                    rhs=vaug[:, h * HD : (h + 1) * HD],
                                    start=(t == 0 and g in (0, 8)),
                                    stop=(t == NT - 1 and g in (7, 15)),
                                    skip_group_check=True,
                                )
                # div partial: reuse the tail of set1 (fully read by exp);
                # next t's set1 score MMs see the WAR dep via the pool buffer.
                dacc = cur_sets[t % 2][:, 4 * N - 16 : 4 * N]
                for h in range(H):
                    for ncn in range(2):
                        g = h * 2 + ncn
                        s0 = (h // 4) * 1024 + ((h % 2) * 2 + (h % 4) // 2) * N
                        lhsT = et[:, s0 + ncn * 128 : s0 + ncn * 128 + 128]
                        nc.tensor.matmul(
                            dacc[:, g : g + 1],
                            lhsT=lhsT,
                            rhs=rbf[:, h : h + 1],
                            start=(g == 0),
                            stop=(g == 15),
                            skip_group_check=True,
                        )
                nc.vector.tensor_add(divs[:], divs[:], dacc)

            # ---------------- Stage D: epilogue ----------------
            dm = spool.tile([128, 16], f32, tag="dm")
            nc.vector.tensor_scalar_max(dm[:], divs[:], 1.0)
            rdiv = spool.tile([128, 16], f32, tag="rdiv")
            nc.vector.reciprocal(rdiv[:], dm[:])
            for ncn in range(2):
                osb = opool.tile([128, C], f32, tag="osb")
                for h in range(H):
                    g = h * 2 + ncn
                    nc.vector.tensor_scalar_mul(
                        osb[:, h * HD : (h + 1) * HD],
                        oacc[:, g * HD : (g + 1) * HD],
                        rdiv[:, g : g + 1],
                    )
                nc.sync.dma_start(out_ap[b, ncn * 128 : (ncn + 1) * 128, :], osb[:])
                nc.sync.dma_start(out2_ap[b, ncn * 128 : (ncn + 1) * 128, :], osb[:])

    nc.compile()
    return nc


def _get_nc():
    with _lock:
        if "nc" not in _cache:
            _cache["nc"] = _build()
        return _cache["nc"]


def kernel(query, key, Wq, Wk):
    from concourse.bass_utils import run_bass_kernel_spmd

    nc = _get_nc()
    query = np.ascontiguousarray(query, dtype=np.float32)
    key = np.ascontiguousarray(key, dtype=np.float32)
    Wq = np.ascontiguousarray(Wq, dtype=np.float32)
    Wk = np.ascontiguousarray(Wk, dtype=np.float32)
    in_maps = [
        {
            "query": query[c * BL : (c + 1) * BL],
            "key": key[c * BL : (c + 1) * BL],
            "Wq": Wq,
            "Wk": Wk,
        }
        for c in range(NCORES)
    ]
    res = run_bass_kernel_spmd(nc, in_maps, core_ids=list(range(NCORES)))
    out = np.concatenate([r["out"] for r in res.results], axis=0)
    out_style = np.concatenate([r["out_style"] for r in res.results], axis=0)
    return out, out_style



# revision 5
# speedup vs baseline: 1.0084x; 1.0084x over previous
"""Trainium2 Bass kernel for nn_AssignAttention (softmax over the query axis).

Math (per batch b):
  q = (query @ Wq)  [N, C] -> heads [N, H, hd]
  k = (key   @ Wk)  [S, C] -> heads [S, H, hd]
  raw[h, n, s] = (q_h @ k_h^T) * hd^-0.5
  attn = softmax(raw, axis=n)                  # normalize over queries, per (h, s)
  attn = attn / max(sum_s attn, 1)             # clamp-normalize over s, per (h, n)
  out[n, h*hd:  ] = sum_s attn[h, n, s] * key[s, h*hd: (h+1)*hd]
  returns (out, out_style) with out_style == out

Distribution: data-parallel over B=16 across 8 NeuronCores (2 batches/core).

v4 structure (vs v2 baseline):
  - key/query are pre-transposed and pre-cast to bf16 on the host; the kernel
    receives key (natural), keyT, qT, so there are NO PE transposes and no
    SWDGE cast-DMAs on chip.  All DMAs ride the HWDGE (sync) queue.
  - out matmuls flipped: stationary = [v_h * (1/D) | 1/D] (65 cols, 54ns
    LDWEIGHTS) with moving = e_h [s,128 x n,256].  The 65th stationary column
    makes PSUM row 64 accumulate div = sum_s attn -- the 512 separate div
    matmuls + LDWEIGHTS of v2 are gone.
  - epilogue transposes oaccT [65, n] back with the div row riding along,
    then scales by 1/max(div,1) per n.
  - den (softmax denominator) split: half on DVE, half on GpSimd.
  - kproj PSUM->SBUF copies split DVE/scalar; scores PSUM tiles double as
    kproj/qproj accumulators (PSUM budget: 2x2 score banks + 4 oacc banks).
"""

import threading

import numpy as np

B, N, S, C, H = 16, 256, 4096, 512, 8
HD = C // H
NCORES = 8
BL = B // NCORES  # batches per core
SCALE = float(HD) ** -0.5

_cache = {}
_lock = threading.Lock()


def _build():
    from contextlib import ExitStack, nullcontext

    import concourse.bass as bass
    import concourse.tile as tile
    from concourse import bacc, mybir
    from concourse.masks import make_identity

    f32 = mybir.dt.float32
    bf16 = mybir.dt.bfloat16

    nc = bacc.Bacc(
        "TRN2",
        target_bir_lowering=False,
        debug=False,
        enable_asserts=False,
        num_devices=NCORES,
    )
    kn_ap = nc.dram_tensor("key_n", [BL, S, C], bf16, kind="ExternalInput").ap()
    kt_ap = nc.dram_tensor("key_t", [BL, C, S], bf16, kind="ExternalInput").ap()
    qt_ap = nc.dram_tensor("q_t", [BL, C, N], bf16, kind="ExternalInput").ap()
    wq_ap = nc.dram_tensor("Wq_b", [C, C], bf16, kind="ExternalInput").ap()
    wk_ap = nc.dram_tensor("Wk_b", [C, C], bf16, kind="ExternalInput").ap()
    out_ap = nc.dram_tensor("out", [BL, N, C], f32, kind="ExternalOutput").ap()

    NT = S // 128          # 32 s-tiles of 128
    NJ = S // 512          # 8 macro chunks of 512 rows
    NCK = C // 128         # 4 c_in chunks
    NM = C // 128          # 4 c_out chunks
    VW = HD + 2            # 66: per-head stride in vaug (64 v + 1 recip + 1 pad)

    with tile.TileContext(nc) as tc, ExitStack() as ctx:
        const = ctx.enter_context(tc.tile_pool(name="const", bufs=1))
        # weights, bf16, layout [c_in_chunk(part=128), k*C + c_out]
        wq_bf = const.tile([128, NCK * C], bf16)
        wk_bf = const.tile([128, NCK * C], bf16)
        nc.sync.dma_start(
            wq_bf[:].rearrange("p (k c) -> p k c", k=NCK),
            wq_ap.rearrange("(k p) c -> p k c", k=NCK),
        )
        nc.sync.dma_start(
            wk_bf[:].rearrange("p (k c) -> p k c", k=NCK),
            wk_ap.rearrange("(k p) c -> p k c", k=NCK),
        )
        identf = const.tile([128, 128], f32)
        make_identity(nc, identf[:])

        # SBUF pools
        kb_pool = ctx.enter_context(tc.tile_pool(name="kb", bufs=1))
        ktt_pool = ctx.enter_context(tc.tile_pool(name="ktt", bufs=2))
        qpool = ctx.enter_context(tc.tile_pool(name="qpool", bufs=2))
        ktpj_pool = ctx.enter_context(tc.tile_pool(name="ktpj", bufs=1))
        epool = ctx.enter_context(tc.tile_pool(name="epool", bufs=3))
        spool = ctx.enter_context(tc.tile_pool(name="spool", bufs=4))
        opool = ctx.enter_context(tc.tile_pool(name="opool", bufs=2))

        # PSUM pools (8 banks: sc 2x2 + oacc 4).  kproj/qproj accumulators and
        # the epilogue transposes allocate from sc_pool (same banks, WAR-cycled).
        sc_pool = ctx.enter_context(tc.tile_pool(name="sc", bufs=1, space="PSUM"))
        oacc_pool = ctx.enter_context(tc.tile_pool(name="oacc", bufs=1, space="PSUM"))

        for b in range(BL):
            # ---------------- Stage A: q path ----------------
            qts = qpool.tile([128, NCK * N], bf16, tag="qts")
            nc.sync.dma_start(
                qts[:].rearrange("p (k n) -> p k n", k=NCK),
                qt_ap[b].rearrange("(k p) n -> p k n", k=NCK),
            )
            # q projection (transposed out): qtp [c_out(part by chunk m), n]
            qtp = qpool.tile([128, NM * N], bf16, tag="qtp")
            for m in range(NM):
                pq = sc_pool.tile([128, 4 * N], f32, tag=f"sc{m % 2}")
                for k in range(NCK):
                    nc.tensor.matmul(
                        pq[:, :N],
                        lhsT=wq_bf[:, k * C + m * 128 : k * C + (m + 1) * 128],
                        rhs=qts[:, k * N : (k + 1) * N],
                        start=(k == 0),
                        stop=(k == NCK - 1),
                    )
                nc.vector.tensor_copy(qtp[:, m * N : (m + 1) * N], pq[:, :N])

            # ---------------- Stage B: k path ----------------
            kb = kb_pool.tile([128, NT * C], bf16, tag="kb")  # natural [s, c] (= V)
            ktpj = ktpj_pool.tile([128, NM * S], bf16, tag="ktpj")  # kT proj [c_out, s]
            for j in range(NJ):
                # natural key rows (v) for this 512-row window
                nc.sync.dma_start(
                    kb[:, 4 * j * C : 4 * (j + 1) * C].rearrange(
                        "p (t c) -> p t c", t=4
                    ),
                    kn_ap[b, j * 512 : (j + 1) * 512, :].rearrange(
                        "(t p) c -> p t c", t=4
                    ),
                )
                # host-transposed keyT columns for this window
                ktt = ktt_pool.tile([128, NCK * 512], bf16, tag="ktt")
                nc.sync.dma_start(
                    ktt[:].rearrange("p (k s) -> p k s", k=NCK),
                    kt_ap[b][:, j * 512 : (j + 1) * 512].rearrange(
                        "(k p) s -> p k s", k=NCK
                    ),
                )
                # k projection, transposed output [c_out(part), s]
                for m in range(NM):
                    pk = sc_pool.tile([128, 4 * N], f32, tag=f"sc{m % 2}")
                    for k in range(NCK):
                        nc.tensor.matmul(
                            pk[:, :512],
                            lhsT=wk_bf[:, k * C + m * 128 : k * C + (m + 1) * 128],
                            rhs=ktt[:, k * 512 : (k + 1) * 512],
                            start=(k == 0),
                            stop=(k == NCK - 1),
                        )
                    if m % 2 == 0:
                        nc.vector.tensor_copy(
                            ktpj[:, m * S + j * 512 : m * S + (j + 1) * 512],
                            pk[:, :512],
                        )
                    else:
                        nc.scalar.copy(
                            ktpj[:, m * S + j * 512 : m * S + (j + 1) * 512],
                            pk[:, :512],
                        )

            # ---------------- Stage C: attention ----------------
            # oacc: per head h an accumulator [65, 256] at free offset h*N;
            # bank h//2.  Row 64 accumulates div (the 65th stationary column).
            oacc = oacc_pool.tile([128, H * N], f32, tag="oacc")

            # scores for tile t: two 2-bank PSUM sets (4 heads each).
            # Within a set, each bank holds a same-parity head pair
            # ((0,2)/(1,3)): mixed row-groups in one bank is a device crash.
            # Issue order (0,1,2,3) makes consecutive (even,odd) MMs hit
            # different banks AND row groups -> they run concurrently.
            def scores(t):
                sets = []
                for half in range(2):
                    scp = sc_pool.tile([128, 4 * N], f32, tag=f"sc{half}")
                    sets.append(scp)
                    for hh in range(4):
                        h = half * 4 + hh
                        m, hp = h // 2, (h % 2) * 64
                        s0 = ((hh % 2) * 2 + hh // 2) * N
                        nc.tensor.matmul(
                            scp[:, s0 : s0 + N],
                            lhsT=ktpj[
                                hp : hp + 64, m * S + t * 128 : m * S + t * 128 + 128
                            ],
                            rhs=qtp[hp : hp + 64, m * N : (m + 1) * N],
                            start=True,
                            stop=True,
                        )
                return sets

            prev_sets = scores(0)
            for t in range(NT):
                cur_sets = prev_sets
                if t + 1 < NT:
                    prev_sets = scores(t + 1)
                et = epool.tile([128, H * N], bf16, tag="et")
                den = spool.tile([128, H], f32, tag="den")
                rt = spool.tile([128, H], f32, tag="rt")
                rbf = spool.tile([128, H], bf16, tag="rbf")
                vaug = spool.tile([128, H * VW], bf16, tag="vaug")
                crit = (
                    tc.tile_critical()
                    if (t == 0 or t == NT - 1)
                    else nullcontext()
                )
                for half in range(2):
                    h0 = half * 4
                    nc.scalar.activation(
                        et[:, half * 1024 : (half + 1) * 1024],
                        cur_sets[half][:],
                        mybir.ActivationFunctionType.Exp,
                        scale=SCALE,
                    )
                    # den[s, slot] = sum_n e; slot-ordered like the sc layout.
                    # tensor_scalar+accum_out runs at a higher DVE perf mode
                    # than tensor_reduce (which only has a 1x uop).
                    etsink = spool.tile([128, N], bf16, tag="etsink")
                    for hh in range(4):
                        nc.vector.tensor_scalar(
                            etsink[:],
                            et[:, half * 1024 + hh * N : half * 1024 + (hh + 1) * N],
                            1.0,
                            0.0,
                            mybir.AluOpType.mult,
                            mybir.AluOpType.add,
                            accum_out=den[:, h0 + hh : h0 + hh + 1],
                        )
                    nc.vector.reciprocal(rt[:, h0 : h0 + 4], den[:, h0 : h0 + 4])
                    # rt is slot-ordered; rbf is head-ordered (the slot
                    # permutation swaps the middle two of each group of 4).
                    nc.vector.tensor_copy(
                        rbf[:, h0 : h0 + 4],
                        rt[:, h0 : h0 + 4]
                        .rearrange("p (a b) -> p a b", a=2, b=2)
                        .transpose([0, 2, 1]),
                    )
                    # vaug[s, h, 0:64] = v[s, h, :] * (1/D[s, h])
                    veng = nc.vector if half == 0 else nc.gpsimd
                    veng.tensor_tensor(
                        vaug[:].rearrange("p (h c) -> p h c", c=VW)[
                            :, h0 : h0 + 4, 0:HD
                        ],
                        kb[:, t * C + h0 * HD : t * C + (h0 + 4) * HD].rearrange(
                            "p (h c) -> p h c", h=4
                        ),
                        rbf[:, h0 : h0 + 4, None].broadcast_to((128, 4, HD)),
                        mybir.AluOpType.mult,
                    )
                # vaug[s, h, 64] = 1/D[s, h]  (the div column)
                nc.vector.tensor_copy(
                    vaug[:].rearrange("p (h c) -> p h c", c=VW)[:, :, HD : HD + 1],
                    rbf[:, :, None],
                )
                # out matmuls: stationary = vaug_h [128s, 65], moving = e_h
                # [128s, 256n] -> oaccT[h] [65, 256] += ...  One start per
                # PSUM bank (h even), one stop per bank (h odd).
                with crit:
                    for half in range(2):
                        for hh in range(4):
                            h = half * 4 + hh
                            s0 = half * 1024 + ((hh % 2) * 2 + hh // 2) * N
                            nc.tensor.matmul(
                                oacc[0:65, h * N : (h + 1) * N],
                                lhsT=vaug[:, h * VW : h * VW + HD + 1],
                                rhs=et[:, s0 : s0 + N],
                                start=(t == 0 and h % 2 == 0),
                                stop=(t == NT - 1 and h % 2 == 1),
                                skip_group_check=True,
                            )

            # ---------------- Stage D: epilogue ----------------
            # transpose oaccT[h] -> [n, 65] per n-chunk; col 64 is div[n].
            for ncn in range(2):
                osb = opool.tile([128, C], f32, tag="osb")
                for half in range(2):
                    osc = spool.tile([65, 4 * N], f32, tag=f"osc{half}")
                    tp = sc_pool.tile([128, 4 * N], f32, tag=f"sc{half}")
                    for hh in range(4):
                        h = half * 4 + hh
                        nc.vector.tensor_copy(
                            osc[:, hh * N : (hh + 1) * N],
                            oacc[0:65, h * N : (h + 1) * N],
                        )
                        nc.tensor.transpose(
                            tp[:, hh * 65 : hh * 65 + 65],
                            osc[0:65, hh * N + ncn * 128 : hh * N + ncn * 128 + 128],
                            identf[0:65, 0:65],
                        )
                    for hh in range(4):
                        h = half * 4 + hh
                        dm = spool.tile([128, 2], f32, tag="dm")
                        nc.vector.tensor_scalar_max(
                            dm[:, 0:1], tp[:, hh * 65 + 64 : hh * 65 + 65], 1.0
                        )
                        nc.vector.reciprocal(dm[:, 1:2], dm[:, 0:1])
                        nc.vector.tensor_scalar_mul(
                            osb[:, h * HD : (h + 1) * HD],
                            tp[:, hh * 65 : hh * 65 + HD],
                            dm[:, 1:2],
                        )
                nc.sync.dma_start(out_ap[b, ncn * 128 : (ncn + 1) * 128, :], osb[:])

    nc.compile()
    return nc


def _get_nc():
    with _lock:
        if "nc" not in _cache:
            _cache["nc"] = _build()
        return _cache["nc"]


def _prep_core_inputs(query, key, Wq, Wk):
    import ml_dtypes

    bf = ml_dtypes.bfloat16
    key_n = np.ascontiguousarray(key.astype(bf))                      # [B, S, C]
    key_t = np.ascontiguousarray(key.transpose(0, 2, 1).astype(bf))   # [B, C, S]
    q_t = np.ascontiguousarray(query.transpose(0, 2, 1).astype(bf))   # [B, C, N]
    wq_b = np.ascontiguousarray(Wq.astype(bf))
    wk_b = np.ascontiguousarray(Wk.astype(bf))
    return [
        {
            "key_n": key_n[c * BL : (c + 1) * BL],
            "key_t": key_t[c * BL : (c + 1) * BL],
            "q_t": q_t[c * BL : (c + 1) * BL],
            "Wq_b": wq_b,
            "Wk_b": wk_b,
        }
        for c in range(NCORES)
    ]


def kernel(query, key, Wq, Wk):
    from concourse.bass_utils import run_bass_kernel_spmd

    nc = _get_nc()
    query = np.ascontiguousarray(query, dtype=np.float32)
    key = np.ascontiguousarray(key, dtype=np.float32)
    Wq = np.ascontiguousarray(Wq, dtype=np.float32)
    Wk = np.ascontiguousarray(Wk, dtype=np.float32)
    in_maps = _prep_core_inputs(query, key, Wq, Wk)
    res = run_bass_kernel_spmd(nc, in_maps, core_ids=list(range(NCORES)))
    out = np.concatenate([r["out"] for r in res.results], axis=0)
    return out, out
